# revision 57
# baseline (speedup 1.0000x reference)
"""Decoupled Contrastive Loss on 8 Trainium2 NeuronCores.

Strategy (data-parallel row slabs, identical SPMD program, per-core np.roll):

Host:
  - f64 row norms; normalize, scale by FP8_SCALE, cast to e4m3 fp8.
  - pack fp8 b-row pairs into uint16 words laid out so that a single
    DMA-xbar u16 transpose lands the data in SBUF as a legal DoubleRow
    fp8 access pattern [Ki, Ko(step%16==0), b] per kp plane:
      P[w, c], w = ko*(R/2) + bp, c = kp*128 + ki,
      word = (f8 x[2bp, d], f8 x[2bp+1, d]), d = kp*256 + ko*128 + ki.
  - per-core np.roll so each core sees its slab at rows 0:1024.
  - mask-weighted raw-sim sums via group-sum identities (as in reference).

Device (per core):
  - DMA-xbar transposes (no PE transposes, no DVE casts, no normalizes).
  - cross-modal pass: slab x full sim via fp8 DoubleRow matmuls into
    2048-wide psum; fused exp (+1/T scale) on ACT with per-row accumulation;
    column accumulation on DVE (bf16).
  - intra-modal passes exploit symmetry: distances 1..8 plus triangular
    masked diagonal tile; d=8 contributes row-side only.

Host combine: per-core partials in f64, assemble the scalar loss.
"""

import numpy as np

TEMPERATURE = 0.07
LAMBDA_V = 0.5
LAMBDA_T = 0.5
B, D = 8192, 512
NC_CORES = 8
SLAB = B // NC_CORES      # 1024
MB = 128
NB = 512
NMB = SLAB // MB          # 8 m-blocks
NNB = B // NB             # 16 n-blocks
VROWS = 10 * NB           # 5120 v rows needed (windows 0..9)
INV_T = 1.0 / TEMPERATURE
FP8_SCALE = 16.0
INV_TS = INV_T / (FP8_SCALE * FP8_SCALE)

_BUILT = None


def _build():
    import concourse.bacc as bacc
    import concourse.tile as tile
    from concourse import mybir
    from contextlib import ExitStack

    f32 = mybir.dt.float32
    bf16 = mybir.dt.bfloat16
    u16 = mybir.dt.uint16
    f8 = mybir.dt.float8e4
    DR = mybir.MatmulPerfMode.DoubleRow
    Exp = mybir.ActivationFunctionType.Exp
    add = mybir.AluOpType.add
    mult = mybir.AluOpType.mult
    AxX = mybir.AxisListType.X

    nc = bacc.Bacc("TRN2", target_bir_lowering=False, debug=False,
                   num_devices=NC_CORES)

    pt_in = nc.dram_tensor("pt", [B, 256], u16, kind="ExternalInput")
    pv_in = nc.dram_tensor("pv", [VROWS, 256], u16, kind="ExternalInput")
    # doubled masks [tri_a | tri_a] so one 1024-wide op covers a (v,t) pair
    tri_in = nc.dram_tensor("tri", [MB, 4, 2 * NB], bf16,
                            kind="ExternalInput")

    rp_sim_out = nc.dram_tensor("rp_sim", [MB, NMB, 5], f32,
                                kind="ExternalOutput")
    ca_sim_out = nc.dram_tensor("ca_sim", [4, MB, 4 * NB], bf16,
                                kind="ExternalOutput")
    rp_v_out = nc.dram_tensor("rp_v", [MB, NMB, 2], f32, kind="ExternalOutput")
    rp_t_out = nc.dram_tensor("rp_t", [MB, NMB, 2], f32, kind="ExternalOutput")
    rd_v_out = nc.dram_tensor("rd_v", [MB, NMB], f32, kind="ExternalOutput")
    rd_t_out = nc.dram_tensor("rd_t", [MB, NMB], f32, kind="ExternalOutput")
    ca_v_out = nc.dram_tensor("ca_v", [MB, 9 * NB], bf16, kind="ExternalOutput")
    ca_t_out = nc.dram_tensor("ca_t", [MB, 9 * NB], bf16, kind="ExternalOutput")

    with tile.TileContext(nc) as tc:
        with ExitStack() as ctx:
            singles = ctx.enter_context(tc.tile_pool(name="singles", bufs=1))
            etp = ctx.enter_context(tc.tile_pool(name="etp", bufs=6))
            psumw = ctx.enter_context(
                tc.tile_pool(name="psumw", bufs=2, space="PSUM"))

            # ---- prime the ACT exp table immediately (2.7us, overlaps DMA) --
            prime = singles.tile([MB, 8], f32, tag="prime", name="prime")
            nc.vector.memset(prime[:], 0.0)
            primeo = singles.tile([MB, 8], f32, tag="primeo", name="primeo")
            nc.scalar.activation(primeo[:], prime[:], Exp, scale=1.0)

            tri_sb = singles.tile([MB, 4, 2 * NB], bf16, tag="tri",
                                  name="tri_sb")

            # ---- transposed fp8 tiles via DMA-xbar u16 transposes ----
            # t: 8 chunk tiles of 1024 b-rows each: [128, 2(kp), 1024 words];
            # f8 byte layout per chunk/kp plane: ko*1024 + b_local
            tt = [singles.tile([MB, 2, 1024], u16, tag=f"tt{c}", name=f"tt{c}")
                  for c in range(8)]
            # v: 5 chunk tiles of 1024 b-rows (windows 0..9)
            vt = [singles.tile([MB, 2, 1024], u16, tag=f"vt{c}", name=f"vt{c}")
                  for c in range(5)]

            def xpose_chunk(dst, src, src_rows, c, eng=None):
                # chunk c covers b in [1024c, 1024c+1024); bp in [512c, +512)
                half = src_rows // 2
                for ko in range(2):
                    (eng or nc.sync).dma_start(
                        out=dst[:, :, 512 * ko:512 * (ko + 1)],
                        in_=src[half * ko + 512 * c:half * ko + 512 * (c + 1), :],
                        transpose=True)

            # transposes serialize on the shared xbar: order by criticality.
            # t0 unblocks the first cross half-group, v0 the lhsT slab.
            xpose_chunk(tt[0], pt_in, B, 0)
            xpose_chunk(vt[0], pv_in, VROWS, 0, eng=nc.scalar)
            nc.scalar.dma_start(out=tri_sb[:], in_=tri_in[:])
            warm = singles.tile([MB, 1], bf16, tag="warm", name="warm")
            nc.vector.tensor_copy(warm[:], tri_sb[:, 0, 0:1])
            xpose_chunk(tt[1], pt_in, B, 1)
            for c in range(2, 8):
                xpose_chunk(tt[c], pt_in, B, c)
            for c in range(1, 5):
                xpose_chunk(vt[c], pv_in, VROWS, c)

            def dr_ap(tiles, kp, b0, width):
                # [Ki, Ko(step 512), width] f8 from chunk tile b0..b0+width
                ctile = tiles[b0 // 1024]
                off = b0 % 1024
                return ctile[:].bitcast(f8)[:, kp, :].rearrange(
                    "k (ko b) -> k ko b", ko=2)[:, :, off:off + width]

            def mm_wide(W, lhs_tiles, m, rhs_tiles, nblk0, nblks):
                # W[:, 512j:...] = sim(m-block, n-blocks nblk0..nblk0+nblks-1)
                for kp in range(2):
                    for j in range(nblks):
                        nc.tensor.matmul(
                            W[:, NB * j:NB * (j + 1)],
                            lhsT=dr_ap(lhs_tiles, kp, MB * m, MB),
                            rhs=dr_ap(rhs_tiles, kp, NB * (nblk0 + j), NB),
                            start=(kp == 0), stop=(kp == 1), perf_mode=DR)

            # ---- shared intra-pass state ----
            # colb is split into A (blocks 0..4) and B (blocks 5..8) so A's
            # output DMA can overlap the final wide groups.
            rp_i = {}
            rd_i = {}
            cbA_i = {}
            cbB_i = {}
            for name in ("v", "t"):
                rp_i[name] = singles.tile([MB, NMB, 2], f32, tag=f"rp_{name}",
                                          name=f"rp_{name}")
                rd_i[name] = singles.tile([MB, NMB], f32, tag=f"rd_{name}",
                                          name=f"rd_{name}")
                cba = singles.tile([MB, 5 * NB], bf16, tag=f"cbA_{name}",
                                   name=f"cbA_{name}")
                nc.gpsimd.memset(cba[:], 0.0)
                cbA_i[name] = cba
                cbb = singles.tile([MB, 4 * NB], bf16, tag=f"cbB_{name}",
                                   name=f"cbB_{name}")
                nc.gpsimd.memset(cbb[:], 0.0)
                cbB_i[name] = cbb

            def emit_diag_quad(m):
                # four diagonal tiles (v,t x m,m+1; m even) share ONE full
                # psum tile and one 2048-wide exp; tri_sb[:, m%4 : m%4+2, :]
                # is exactly the [tri_a|tri_a|tri_a1|tri_a1] quad mask.
                G = m // 4
                slots = ((0, vt, "v", m), (1, tt, "t", m),
                         (2, vt, "v", m + 1), (3, tt, "t", m + 1))
                Wd = psumw.tile([MB, 4 * NB], f32, tag="W", name=f"Wdq{m}")
                for j, xtiles, name, dm in slots:
                    for kp in range(2):
                        nc.tensor.matmul(
                            Wd[:, j * NB:(j + 1) * NB],
                            lhsT=dr_ap(xtiles, kp, MB * dm, MB),
                            rhs=dr_ap(xtiles, kp, NB * G, NB),
                            start=(kp == 0), stop=(kp == 1), perf_mode=DR)
                etd = etp.tile([MB, 4 * NB], bf16, tag="et", name="etdq")
                nc.scalar.activation(etd[:], Wd[:], Exp, scale=INV_TS)
                em = etp.tile([MB, 4 * NB], bf16, tag="et", name="emq")
                nc.vector.tensor_mul(
                    em[:], etd[:],
                    tri_sb[:, m % 4:m % 4 + 2, :].rearrange(
                        "p a b -> p (a b)"))
                for j, xtiles, name, dm in slots:
                    sl = em[:, j * NB:(j + 1) * NB]
                    nc.vector.tensor_reduce(rd_i[name][:, dm:dm + 1], sl,
                                            axis=AxX, op=add)
                    cba = cbA_i[name]
                    nc.vector.tensor_add(
                        cba[:, G * NB:(G + 1) * NB],
                        cba[:, G * NB:(G + 1) * NB], sl)

            # ---- PE warmup: dummy matmuls during the transpose window so
            # the HAM clock-gate reaches full rate before the first real
            # matmul group (results are never read).
            dummy = singles.tile([MB, 2 * NB], bf16, tag="dummy",
                                 name="dummy")
            nc.vector.memset(dummy[:], 0.0)
            Wu = psumw.tile([MB, 4 * NB], f32, tag="W", name="Wu")
            for i in range(16):
                nc.tensor.matmul(Wu[:, 0:NB], lhsT=dummy[:, 0:MB],
                                 rhs=dummy[:, 0:NB], start=True, stop=True)

            # ---- cross-modal pass (diag pairs interleaved) ----
            # (p=0, m=0) is split into two 2-block halves so the first exp
            # only depends on the t0 + v0 transposes.
            rp_sim = singles.tile([MB, NMB, 5], f32, tag="rp_sim",
                                  name="rp_sim")
            di = 0
            for p in range(4):
                colacc = singles.tile([MB, 4 * NB], bf16, tag=f"cas{p}",
                                      name=f"cas{p}")
                for m in range(NMB):
                    if p == 0 and m == 0:
                        ha = psumw.tile([MB, 4 * NB], f32, tag="W", name="Wa")
                        mm_wide(ha[:, 0:2 * NB], vt, 0, tt, 0, 2)
                        eta = etp.tile([MB, 2 * NB], bf16, tag="eth",
                                       name="eta")
                        nc.scalar.activation(eta[:], ha[:, 0:2 * NB], Exp,
                                             scale=INV_TS,
                                             accum_out=rp_sim[:, 0, 4:5])
                        emit_diag_quad(0); di += 1
                        hb = psumw.tile([MB, 4 * NB], f32, tag="W", name="Wb")
                        mm_wide(hb[:, 0:2 * NB], vt, 0, tt, 2, 2)
                        etb = etp.tile([MB, 2 * NB], bf16, tag="eth",
                                       name="etb")
                        nc.scalar.activation(etb[:], hb[:, 0:2 * NB], Exp,
                                             scale=INV_TS,
                                             accum_out=rp_sim[:, 0, 0:1])
                        nc.vector.tensor_copy(colacc[:, 0:2 * NB], eta[:])
                        nc.vector.tensor_copy(colacc[:, 2 * NB:4 * NB],
                                              etb[:])
                        continue
                    W = psumw.tile([MB, 4 * NB], f32, tag="W", name="Wc")
                    mm_wide(W, vt, m, tt, 4 * p, 4)
                    et = etp.tile([MB, 4 * NB], bf16, tag="et", name="etc")
                    nc.scalar.activation(et[:], W[:], Exp, scale=INV_TS,
                                         accum_out=rp_sim[:, m, p:p + 1])
                    if m == 0:
                        nc.vector.tensor_copy(colacc[:], et[:])
                    else:
                        nc.vector.tensor_add(colacc[:], colacc[:], et[:])
                    # one diag quad (m,m+1 both names) per EIGHT cross
                    # groups: keeps the per-window DVE load under ACT pace
                    if di < 4 and (8 * p + m) % 8 == 1:
                        emit_diag_quad(2 * di); di += 1
                nc.sync.dma_start(out=ca_sim_out[p], in_=colacc[:])
            nc.sync.dma_start(out=rp_sim_out[:], in_=rp_sim[:])
            # rd tiles are final once all diag groups (cross phase) are done
            nc.sync.dma_start(out=rd_v_out[:], in_=rd_i["v"][:])
            nc.sync.dma_start(out=rd_t_out[:], in_=rd_i["t"][:])

            # ---- intra-modal wide passes (clean 2-deep psum ping-pong) ----
            for name, xtiles, rp_out, rd_out, ca_out in (
                    ("v", vt, rp_v_out, rd_v_out, ca_v_out),
                    ("t", tt, rp_t_out, rd_t_out, ca_t_out)):
                rp = rp_i[name]
                cba, cbb = cbA_i[name], cbB_i[name]
                for m in range(NMB):
                    G = m // 4
                    # W0: blocks G+1..G+4 (row+col), W1: G+5..G+8 (col only
                    # for G+5..G+7; block G+8 is row-side only)
                    for h, nb0 in ((0, G + 1), (1, G + 5)):
                        W = psumw.tile([MB, 4 * NB], f32, tag="W", name="Wp")
                        mm_wide(W, xtiles, m, xtiles, nb0, 4)
                        et = etp.tile([MB, 4 * NB], bf16, tag="et",
                                      name="etp_")
                        nc.scalar.activation(et[:], W[:], Exp, scale=INV_TS,
                                             accum_out=rp[:, m, h:h + 1])
                        cw = 4 * NB if h == 0 else 3 * NB
                        # split the column-partial add at the A|B boundary
                        # (A = blocks 0..4, B = blocks 5..8)
                        lo = nb0 * NB
                        hi = lo + cw
                        if lo < 5 * NB:
                            w0 = min(hi, 5 * NB) - lo
                            nc.vector.tensor_add(
                                cba[:, lo:lo + w0], cba[:, lo:lo + w0],
                                et[:, 0:w0])
                        if hi > 5 * NB:
                            b0 = max(lo, 5 * NB)
                            s0 = b0 - lo
                            nc.vector.tensor_add(
                                cbb[:, b0 - 5 * NB:hi - 5 * NB],
                                cbb[:, b0 - 5 * NB:hi - 5 * NB],
                                et[:, s0:s0 + (hi - b0)])
                        if m == NMB - 1 and h == 0:
                            # A is final after the last h0 add; only the
                            # final (t) pass uses the ACT hwdge queue —
                            # mid-kernel it would stall exps behind it
                            if name == "t":
                                # many small DMAs spread across engines
                                for k, eng in ((0, nc.scalar),
                                               (1, nc.sync),
                                               (2, nc.scalar),
                                               (3, nc.sync),
                                               (4, nc.scalar)):
                                    eng.dma_start(
                                        out=ca_out[:, k * NB:(k + 1) * NB],
                                        in_=cba[:, k * NB:(k + 1) * NB])
                            else:
                                nc.sync.dma_start(out=ca_out[:, 0:5 * NB],
                                                  in_=cba[:])
                if name == "t":
                    # final pass: split B across engines via small DMAs
                    for k, eng in ((0, nc.sync), (1, nc.scalar),
                                   (2, nc.sync), (3, nc.scalar)):
                        eng.dma_start(
                            out=ca_out[:, (5 + k) * NB:(6 + k) * NB],
                            in_=cbb[:, k * NB:(k + 1) * NB])
                elif False:
                    pass
                else:
                    nc.sync.dma_start(out=ca_out[:, 5 * NB:9 * NB],
                                      in_=cbb[:])
                nc.sync.dma_start(out=rp_out[:], in_=rp[:])

    nc.compile()
    return nc


def _get_nc():
    global _BUILT
    if _BUILT is None:
        _BUILT = _build()
    return _BUILT


def _pack_dr(x8u):
    """x8u: [R, 512] uint8 (fp8 e4m3 bytes), R even.
    Returns [R, 256] '<u2': P[w, c], w = ko*(R/2) + bp, c = kp*128 + ki,
    word = (x[2bp, d], x[2bp+1, d]), d = kp*256 + ko*128 + ki."""
    R = x8u.shape[0]
    Xr = x8u.reshape(R // 2, 2, 2, 2, 128)            # [bp, r, kp, ko, ki]
    Pb = np.ascontiguousarray(Xr.transpose(3, 0, 2, 4, 1))  # [ko,bp,kp,ki,r]
    return Pb.reshape(R, 512).view("<u2")


def _host_prep(v, t, ids):
    v64, t64 = v.astype(np.float64), t.astype(np.float64)
    rnv = (1.0 / np.sqrt((v64 * v64).sum(1))).astype(np.float32)
    rnt = (1.0 / np.sqrt((t64 * t64).sum(1))).astype(np.float32)
    vn = v * rnv[:, None]
    tn = t * rnt[:, None]

    cnt = np.bincount(ids, minlength=2048)[ids].astype(np.float64)
    npos = max(int((cnt - 1).sum()), 1)

    order = np.argsort(ids, kind="stable")
    ids_s = ids[order]
    starts = np.r_[0, 1 + np.flatnonzero(np.diff(ids_s))]
    Vg = np.add.reduceat(vn[order].astype(np.float64), starts, axis=0)
    Tg = np.add.reduceat(tn[order].astype(np.float64), starts, axis=0)
    return dict(
        vn=vn, tn=tn, cnt=cnt, npos=npos,
        sig_vt=(Vg * Tg).sum(), sig_vv=(Vg * Vg).sum(), sig_tt=(Tg * Tg).sum(),
        diag_vv=(vn.astype(np.float64) ** 2).sum(),
        diag_tt=(tn.astype(np.float64) ** 2).sum())


def _trimask():
    import ml_dtypes
    m = np.zeros((MB, 4, 2 * NB), dtype=ml_dtypes.bfloat16)
    cols = np.arange(NB)[None, :]
    rows = np.arange(MB)[:, None]
    for a in range(4):
        tri = ((cols - 128 * a) > rows).astype(ml_dtypes.bfloat16)
        m[:, a, 0:NB] = tri
        m[:, a, NB:2 * NB] = tri
    return m


def run(v, t, ids, trace=False):
    """Run device + host combine. Returns (loss, BassKernelResults)."""
    import ml_dtypes
    from concourse.bass_utils import run_bass_kernel_spmd

    v = np.ascontiguousarray(np.asarray(v, dtype=np.float32))
    t = np.ascontiguousarray(np.asarray(t, dtype=np.float32))
    ids = np.asarray(ids).astype(np.int64)

    prep = _host_prep(v, t, ids)
    tri = _trimask()

    v8 = (prep["vn"] * FP8_SCALE).astype(ml_dtypes.float8_e4m3fn)
    t8 = (prep["tn"] * FP8_SCALE).astype(ml_dtypes.float8_e4m3fn)
    v8u = np.ascontiguousarray(v8.view(np.uint8))
    t8u = np.ascontiguousarray(t8.view(np.uint8))

    in_maps = []
    for c in range(NC_CORES):
        s = SLAB * c
        tv = np.roll(t8u, -s, axis=0)
        vv = np.roll(v8u, -s, axis=0)[:VROWS]
        in_maps.append({
            "pt": _pack_dr(tv),
            "pv": _pack_dr(np.ascontiguousarray(vv)),
            "tri": tri,
        })

    nc = _get_nc()
    res = run_bass_kernel_spmd(
        nc, in_maps, core_ids=list(range(NC_CORES)), trace=trace)

    loss = _combine(res.results, prep)
    return loss, res


def _combine(results, prep):
    cnt, npos = prep["cnt"], prep["npos"]
    rowsum_sim = np.zeros(B)
    S_col = np.zeros(B)
    acc = {name: dict(row=np.zeros(B), col=np.zeros(B))
           for name in ("v", "t")}
    for c in range(NC_CORES):
        r = results[c]
        s = SLAB * c
        gsl = slice(s, s + SLAB)
        rps = r["rp_sim"].astype(np.float64)          # [128, 8, 5]
        # slots 0..3 are written for every m; slot 4 only by the (p=0, m=0)
        # bootstrap half-group — the rest of slot 4 is never written
        rsum = rps[:, :, 0:4].sum(axis=2)
        rsum[:, 0] += rps[:, 0, 4]
        rowsum_sim[gsl] += rsum.T.reshape(SLAB)
        cas = r["ca_sim"].astype(np.float64)          # [4, 128, 2048]
        S_col += np.roll(cas.sum(axis=1).reshape(B), s)
        for name in ("v", "t"):
            rp = r[f"rp_{name}"].astype(np.float64)   # [128, 8, 2]
            rdg = r[f"rd_{name}"].astype(np.float64)  # [128, 8]
            acc[name]["row"][gsl] += (rp.sum(axis=2) + rdg).T.reshape(SLAB)
            ca = r[f"ca_{name}"].astype(np.float64)   # [128, 9*512]
            colfull = np.zeros(B)
            colfull[:9 * NB] = ca.sum(axis=0)
            acc[name]["col"] += np.roll(colfull, s)

    lse_row = np.log(rowsum_sim)
    lse_col = np.log(S_col)
    v2t = (cnt @ lse_row - prep["sig_vt"] * INV_T) / npos
    t2v = (cnt @ lse_col - prep["sig_vt"] * INV_T) / npos

    inst = {}
    for name, sig, diag_raw in (("v", prep["sig_vv"], prep["diag_vv"]),
                                ("t", prep["sig_tt"], prep["diag_tt"])):
        a = acc[name]
        rs = a["row"] + a["col"]
        lse = np.log(rs)
        inst[name] = ((cnt - 1) @ lse - (sig - diag_raw) * INV_T) / npos

    total = 0.5 * (v2t + t2v) + LAMBDA_V * inst["v"] + LAMBDA_T * inst["t"]
    return np.float32(total)


def kernel(vision_features, text_features, match_ids):
    loss, _ = run(vision_features, text_features, match_ids)
    return np.array(loss, dtype=np.float32)


# revision 59
# speedup vs baseline: 1.1851x; 1.1851x over previous
"""Decoupled Contrastive Loss on 8 Trainium2 NeuronCores.

Strategy (data-parallel row slabs, identical SPMD program, per-core np.roll):

Host:
  - f64 row norms; normalize, scale by FP8_SCALE, cast to e4m3 fp8.
  - pack fp8 b-row pairs into uint16 words laid out so that a single
    DMA-xbar u16 transpose lands the data in SBUF as a legal DoubleRow
    fp8 access pattern [Ki, Ko(step%16==0), b] per kp plane:
      P[w, c], w = ko*(R/2) + bp, c = kp*128 + ki,
      word = (f8 x[2bp, d], f8 x[2bp+1, d]), d = kp*256 + ko*128 + ki.
  - per-core np.roll so each core sees its slab at rows 0:1024.
  - mask-weighted raw-sim sums via group-sum identities (as in reference).

Device (per core):
  - DMA-xbar transposes (no PE transposes, no DVE casts, no normalizes).
  - cross-modal pass: slab x full sim via fp8 DoubleRow matmuls into
    2048-wide psum; fused exp (+1/T scale) on ACT with per-row accumulation;
    column accumulation on DVE (bf16).
  - intra-modal passes exploit symmetry: distances 1..8 plus triangular
    masked diagonal tile; d=8 contributes row-side only.

Host combine: per-core partials in f64, assemble the scalar loss.
"""

import numpy as np

TEMPERATURE = 0.07
LAMBDA_V = 0.5
LAMBDA_T = 0.5
B, D = 8192, 512
NC_CORES = 8
SLAB = B // NC_CORES      # 1024
MB = 128
NB = 512
NMB = SLAB // MB          # 8 m-blocks
NNB = B // NB             # 16 n-blocks
VROWS = 10 * NB           # 5120 v rows needed (windows 0..9)
INV_T = 1.0 / TEMPERATURE
FP8_SCALE = 16.0
INV_TS = INV_T / (FP8_SCALE * FP8_SCALE)

_BUILT = None


def _build():
    import concourse.bacc as bacc
    import concourse.tile as tile
    from concourse import mybir
    from contextlib import ExitStack

    f32 = mybir.dt.float32
    bf16 = mybir.dt.bfloat16
    u16 = mybir.dt.uint16
    f8 = mybir.dt.float8e4
    DR = mybir.MatmulPerfMode.DoubleRow
    Exp = mybir.ActivationFunctionType.Exp
    add = mybir.AluOpType.add
    mult = mybir.AluOpType.mult
    AxX = mybir.AxisListType.X

    nc = bacc.Bacc("TRN2", target_bir_lowering=False, debug=False,
                   num_devices=NC_CORES)

    pt_in = nc.dram_tensor("pt", [B, 256], u16, kind="ExternalInput")
    pv_in = nc.dram_tensor("pv", [VROWS, 256], u16, kind="ExternalInput")
    # doubled masks [tri_a | tri_a] so one 1024-wide op covers a (v,t) pair
    tri_in = nc.dram_tensor("tri", [MB, 4, 2 * NB], bf16,
                            kind="ExternalInput")

    rp_sim_out = nc.dram_tensor("rp_sim", [MB, NMB, 5], f32,
                                kind="ExternalOutput")
    ca_sim_out = nc.dram_tensor("ca_sim", [4, MB, 4 * NB], bf16,
                                kind="ExternalOutput")
    rp_v_out = nc.dram_tensor("rp_v", [MB, NMB, 2], f32, kind="ExternalOutput")
    rp_t_out = nc.dram_tensor("rp_t", [MB, NMB, 2], f32, kind="ExternalOutput")
    rd_v_out = nc.dram_tensor("rd_v", [MB, NMB], f32, kind="ExternalOutput")
    rd_t_out = nc.dram_tensor("rd_t", [MB, NMB], f32, kind="ExternalOutput")
    ca_v_out = nc.dram_tensor("ca_v", [MB, 9 * NB], bf16, kind="ExternalOutput")
    ca_t_out = nc.dram_tensor("ca_t", [MB, 9 * NB], bf16, kind="ExternalOutput")

    with tile.TileContext(nc) as tc:
        with ExitStack() as ctx:
            singles = ctx.enter_context(tc.tile_pool(name="singles", bufs=1))
            etp = ctx.enter_context(tc.tile_pool(name="etp", bufs=6))
            psumw = ctx.enter_context(
                tc.tile_pool(name="psumw", bufs=2, space="PSUM"))

            # ---- prime the ACT exp table immediately (2.7us, overlaps DMA) --
            prime = singles.tile([MB, 8], f32, tag="prime", name="prime")
            nc.vector.memset(prime[:], 0.0)
            primeo = singles.tile([MB, 8], f32, tag="primeo", name="primeo")
            nc.scalar.activation(primeo[:], prime[:], Exp, scale=1.0)

            tri_sb = singles.tile([MB, 4, 2 * NB], bf16, tag="tri",
                                  name="tri_sb")

            # ---- transposed fp8 tiles via DMA-xbar u16 transposes ----
            # t: 8 chunk tiles of 1024 b-rows each: [128, 2(kp), 1024 words];
            # f8 byte layout per chunk/kp plane: ko*1024 + b_local
            tt = [singles.tile([MB, 2, 1024], u16, tag=f"tt{c}", name=f"tt{c}")
                  for c in range(8)]
            # v: 5 chunk tiles of 1024 b-rows (windows 0..9)
            vt = [singles.tile([MB, 2, 1024], u16, tag=f"vt{c}", name=f"vt{c}")
                  for c in range(5)]

            def xpose_chunk(dst, src, src_rows, c, eng=None):
                # chunk c covers b in [1024c, 1024c+1024); bp in [512c, +512)
                half = src_rows // 2
                for ko in range(2):
                    (eng or nc.sync).dma_start(
                        out=dst[:, :, 512 * ko:512 * (ko + 1)],
                        in_=src[half * ko + 512 * c:half * ko + 512 * (c + 1), :],
                        transpose=True)

            # transposes serialize on the shared xbar: order by criticality.
            # t0 unblocks the first cross half-group, v0 the lhsT slab.
            xpose_chunk(tt[0], pt_in, B, 0)
            xpose_chunk(vt[0], pv_in, VROWS, 0, eng=nc.scalar)
            nc.scalar.dma_start(out=tri_sb[:], in_=tri_in[:])
            warm = singles.tile([MB, 1], bf16, tag="warm", name="warm")
            nc.vector.tensor_copy(warm[:], tri_sb[:, 0, 0:1])
            xpose_chunk(tt[1], pt_in, B, 1)
            for c in range(2, 8):
                xpose_chunk(tt[c], pt_in, B, c)
            for c in range(1, 5):
                xpose_chunk(vt[c], pv_in, VROWS, c)

            def dr_ap(tiles, kp, b0, width):
                # [Ki, Ko(step 512), width] f8 from chunk tile b0..b0+width
                ctile = tiles[b0 // 1024]
                off = b0 % 1024
                return ctile[:].bitcast(f8)[:, kp, :].rearrange(
                    "k (ko b) -> k ko b", ko=2)[:, :, off:off + width]

            def mm_wide(W, lhs_tiles, m, rhs_tiles, nblk0, nblks):
                # W[:, 512j:...] = sim(m-block, n-blocks nblk0..nblk0+nblks-1)
                for kp in range(2):
                    for j in range(nblks):
                        nc.tensor.matmul(
                            W[:, NB * j:NB * (j + 1)],
                            lhsT=dr_ap(lhs_tiles, kp, MB * m, MB),
                            rhs=dr_ap(rhs_tiles, kp, NB * (nblk0 + j), NB),
                            start=(kp == 0), stop=(kp == 1), perf_mode=DR)

            # ---- shared intra-pass state ----
            # colb is split into A (blocks 0..4) and B (blocks 5..8) so A's
            # output DMA can overlap the final wide groups.
            rp_i = {}
            rd_i = {}
            cbA_i = {}
            cbB_i = {}
            for name in ("v", "t"):
                rp_i[name] = singles.tile([MB, NMB, 2], f32, tag=f"rp_{name}",
                                          name=f"rp_{name}")
                rd_i[name] = singles.tile([MB, NMB], f32, tag=f"rd_{name}",
                                          name=f"rd_{name}")
                cba = singles.tile([MB, 5 * NB], bf16, tag=f"cbA_{name}",
                                   name=f"cbA_{name}")
                nc.gpsimd.memset(cba[:], 0.0)
                cbA_i[name] = cba
                cbb = singles.tile([MB, 4 * NB], bf16, tag=f"cbB_{name}",
                                   name=f"cbB_{name}")
                nc.gpsimd.memset(cbb[:], 0.0)
                cbB_i[name] = cbb

            def emit_diag_quad(m):
                # four diagonal tiles (v,t x m,m+1; m even) share ONE full
                # psum tile and one 2048-wide exp; tri_sb[:, m%4 : m%4+2, :]
                # is exactly the [tri_a|tri_a|tri_a1|tri_a1] quad mask.
                G = m // 4
                slots = ((0, vt, "v", m), (1, tt, "t", m),
                         (2, vt, "v", m + 1), (3, tt, "t", m + 1))
                Wd = psumw.tile([MB, 4 * NB], f32, tag="W", name=f"Wdq{m}")
                for j, xtiles, name, dm in slots:
                    for kp in range(2):
                        nc.tensor.matmul(
                            Wd[:, j * NB:(j + 1) * NB],
                            lhsT=dr_ap(xtiles, kp, MB * dm, MB),
                            rhs=dr_ap(xtiles, kp, NB * G, NB),
                            start=(kp == 0), stop=(kp == 1), perf_mode=DR)
                etd = etp.tile([MB, 4 * NB], bf16, tag="et", name="etdq")
                nc.scalar.activation(etd[:], Wd[:], Exp, scale=INV_TS)
                em = etp.tile([MB, 4 * NB], bf16, tag="et", name="emq")
                nc.vector.tensor_mul(
                    em[:], etd[:],
                    tri_sb[:, m % 4:m % 4 + 2, :].rearrange(
                        "p a b -> p (a b)"))
                for j, xtiles, name, dm in slots:
                    sl = em[:, j * NB:(j + 1) * NB]
                    nc.vector.tensor_reduce(rd_i[name][:, dm:dm + 1], sl,
                                            axis=AxX, op=add)
                    cba = cbA_i[name]
                    nc.vector.tensor_add(
                        cba[:, G * NB:(G + 1) * NB],
                        cba[:, G * NB:(G + 1) * NB], sl)

            # ---- PE warmup: dummy matmuls during the transpose window so
            # the HAM clock-gate reaches full rate before the first real
            # matmul group (results are never read).
            dummy = singles.tile([MB, 2 * NB], bf16, tag="dummy",
                                 name="dummy")
            nc.vector.memset(dummy[:], 0.0)
            Wu = psumw.tile([MB, 4 * NB], f32, tag="W", name="Wu")
            for i in range(16):
                nc.tensor.matmul(Wu[:, 0:NB], lhsT=dummy[:, 0:MB],
                                 rhs=dummy[:, 0:NB], start=True, stop=True)

            # ---- cross-modal pass (diag pairs interleaved) ----
            # (p=0, m=0) is split into two 2-block halves so the first exp
            # only depends on the t0 + v0 transposes.
            rp_sim = singles.tile([MB, NMB, 5], f32, tag="rp_sim",
                                  name="rp_sim")
            di = 0
            for p in range(4):
                colacc = singles.tile([MB, 4 * NB], bf16, tag=f"cas{p}",
                                      name=f"cas{p}")
                for m in range(NMB):
                    if p == 0 and m == 0:
                        ha = psumw.tile([MB, 4 * NB], f32, tag="W", name="Wa")
                        mm_wide(ha[:, 0:2 * NB], vt, 0, tt, 0, 2)
                        eta = etp.tile([MB, 2 * NB], bf16, tag="eth",
                                       name="eta")
                        nc.scalar.activation(eta[:], ha[:, 0:2 * NB], Exp,
                                             scale=INV_TS,
                                             accum_out=rp_sim[:, 0, 4:5])
                        emit_diag_quad(0); di += 1
                        hb = psumw.tile([MB, 4 * NB], f32, tag="W", name="Wb")
                        mm_wide(hb[:, 0:2 * NB], vt, 0, tt, 2, 2)
                        etb = etp.tile([MB, 2 * NB], bf16, tag="eth",
                                       name="etb")
                        nc.scalar.activation(etb[:], hb[:, 0:2 * NB], Exp,
                                             scale=INV_TS,
                                             accum_out=rp_sim[:, 0, 0:1])
                        nc.vector.tensor_copy(colacc[:, 0:2 * NB], eta[:])
                        nc.vector.tensor_copy(colacc[:, 2 * NB:4 * NB],
                                              etb[:])
                        continue
                    W = psumw.tile([MB, 4 * NB], f32, tag="W", name="Wc")
                    mm_wide(W, vt, m, tt, 4 * p, 4)
                    et = etp.tile([MB, 4 * NB], bf16, tag="et", name="etc")
                    nc.scalar.activation(et[:], W[:], Exp, scale=INV_TS,
                                         accum_out=rp_sim[:, m, p:p + 1])
                    if m == 0:
                        nc.vector.tensor_copy(colacc[:], et[:])
                    else:
                        nc.vector.tensor_add(colacc[:], colacc[:], et[:])
                    # one diag quad (m,m+1 both names) per EIGHT cross
                    # groups: keeps the per-window DVE load under ACT pace
                    if di < 4 and (8 * p + m) % 8 == 1:
                        emit_diag_quad(2 * di); di += 1
                nc.sync.dma_start(out=ca_sim_out[p], in_=colacc[:])
            nc.sync.dma_start(out=rp_sim_out[:], in_=rp_sim[:])
            # rd tiles are final once all diag groups (cross phase) are done
            nc.sync.dma_start(out=rd_v_out[:], in_=rd_i["v"][:])
            nc.sync.dma_start(out=rd_t_out[:], in_=rd_i["t"][:])

            # ---- intra-modal wide passes (clean 2-deep psum ping-pong) ----
            for name, xtiles, rp_out, rd_out, ca_out in (
                    ("v", vt, rp_v_out, rd_v_out, ca_v_out),
                    ("t", tt, rp_t_out, rd_t_out, ca_t_out)):
                rp = rp_i[name]
                cba, cbb = cbA_i[name], cbB_i[name]
                for m in range(NMB):
                    G = m // 4
                    # W0: blocks G+1..G+4 (row+col), W1: G+5..G+8 (col only
                    # for G+5..G+7; block G+8 is row-side only)
                    for h, nb0 in ((0, G + 1), (1, G + 5)):
                        W = psumw.tile([MB, 4 * NB], f32, tag="W", name="Wp")
                        mm_wide(W, xtiles, m, xtiles, nb0, 4)
                        et = etp.tile([MB, 4 * NB], bf16, tag="et",
                                      name="etp_")
                        nc.scalar.activation(et[:], W[:], Exp, scale=INV_TS,
                                             accum_out=rp[:, m, h:h + 1])
                        cw = 4 * NB if h == 0 else 3 * NB
                        # split the column-partial add at the A|B boundary
                        # (A = blocks 0..4, B = blocks 5..8)
                        lo = nb0 * NB
                        hi = lo + cw
                        if lo < 5 * NB:
                            w0 = min(hi, 5 * NB) - lo
                            nc.vector.tensor_add(
                                cba[:, lo:lo + w0], cba[:, lo:lo + w0],
                                et[:, 0:w0])
                        if hi > 5 * NB:
                            b0 = max(lo, 5 * NB)
                            s0 = b0 - lo
                            nc.vector.tensor_add(
                                cbb[:, b0 - 5 * NB:hi - 5 * NB],
                                cbb[:, b0 - 5 * NB:hi - 5 * NB],
                                et[:, s0:s0 + (hi - b0)])
                        if m == NMB - 1 and h == 0:
                            # A is final after the last h0 add; only the
                            # final (t) pass uses the ACT hwdge queue —
                            # mid-kernel it would stall exps behind it
                            eng = nc.scalar if name == "t" else nc.sync
                            eng.dma_start(out=ca_out[:, 0:5 * NB],
                                          in_=cba[:])
                if name == "t":
                    # final pass: split B across both hwdge queues
                    nc.sync.dma_start(out=ca_out[:, 5 * NB:7 * NB],
                                      in_=cbb[:, 0:2 * NB])
                    nc.scalar.dma_start(out=ca_out[:, 7 * NB:9 * NB],
                                        in_=cbb[:, 2 * NB:4 * NB])
                else:
                    nc.sync.dma_start(out=ca_out[:, 5 * NB:9 * NB],
                                      in_=cbb[:])
                nc.sync.dma_start(out=rp_out[:], in_=rp[:])

    nc.compile()
    return nc


def _get_nc():
    global _BUILT
    if _BUILT is None:
        _BUILT = _build()
    return _BUILT


def _pack_dr(x8u):
    """x8u: [R, 512] uint8 (fp8 e4m3 bytes), R even.
    Returns [R, 256] '<u2': P[w, c], w = ko*(R/2) + bp, c = kp*128 + ki,
    word = (x[2bp, d], x[2bp+1, d]), d = kp*256 + ko*128 + ki."""
    R = x8u.shape[0]
    Xr = x8u.reshape(R // 2, 2, 2, 2, 128)            # [bp, r, kp, ko, ki]
    Pb = np.ascontiguousarray(Xr.transpose(3, 0, 2, 4, 1))  # [ko,bp,kp,ki,r]
    return Pb.reshape(R, 512).view("<u2")


def _host_prep(v, t, ids):
    v64, t64 = v.astype(np.float64), t.astype(np.float64)
    rnv = (1.0 / np.sqrt((v64 * v64).sum(1))).astype(np.float32)
    rnt = (1.0 / np.sqrt((t64 * t64).sum(1))).astype(np.float32)
    vn = v * rnv[:, None]
    tn = t * rnt[:, None]

    cnt = np.bincount(ids, minlength=2048)[ids].astype(np.float64)
    npos = max(int((cnt - 1).sum()), 1)

    order = np.argsort(ids, kind="stable")
    ids_s = ids[order]
    starts = np.r_[0, 1 + np.flatnonzero(np.diff(ids_s))]
    Vg = np.add.reduceat(vn[order].astype(np.float64), starts, axis=0)
    Tg = np.add.reduceat(tn[order].astype(np.float64), starts, axis=0)
    return dict(
        vn=vn, tn=tn, cnt=cnt, npos=npos,
        sig_vt=(Vg * Tg).sum(), sig_vv=(Vg * Vg).sum(), sig_tt=(Tg * Tg).sum(),
        diag_vv=(vn.astype(np.float64) ** 2).sum(),
        diag_tt=(tn.astype(np.float64) ** 2).sum())


def _trimask():
    import ml_dtypes
    m = np.zeros((MB, 4, 2 * NB), dtype=ml_dtypes.bfloat16)
    cols = np.arange(NB)[None, :]
    rows = np.arange(MB)[:, None]
    for a in range(4):
        tri = ((cols - 128 * a) > rows).astype(ml_dtypes.bfloat16)
        m[:, a, 0:NB] = tri
        m[:, a, NB:2 * NB] = tri
    return m


def run(v, t, ids, trace=False):
    """Run device + host combine. Returns (loss, BassKernelResults)."""
    import ml_dtypes
    from concourse.bass_utils import run_bass_kernel_spmd

    v = np.ascontiguousarray(np.asarray(v, dtype=np.float32))
    t = np.ascontiguousarray(np.asarray(t, dtype=np.float32))
    ids = np.asarray(ids).astype(np.int64)

    prep = _host_prep(v, t, ids)
    tri = _trimask()

    v8 = (prep["vn"] * FP8_SCALE).astype(ml_dtypes.float8_e4m3fn)
    t8 = (prep["tn"] * FP8_SCALE).astype(ml_dtypes.float8_e4m3fn)
    v8u = np.ascontiguousarray(v8.view(np.uint8))
    t8u = np.ascontiguousarray(t8.view(np.uint8))

    in_maps = []
    for c in range(NC_CORES):
        s = SLAB * c
        tv = np.roll(t8u, -s, axis=0)
        vv = np.roll(v8u, -s, axis=0)[:VROWS]
        in_maps.append({
            "pt": _pack_dr(tv),
            "pv": _pack_dr(np.ascontiguousarray(vv)),
            "tri": tri,
        })

    nc = _get_nc()
    res = run_bass_kernel_spmd(
        nc, in_maps, core_ids=list(range(NC_CORES)), trace=trace)

    loss = _combine(res.results, prep)
    return loss, res


def _combine(results, prep):
    cnt, npos = prep["cnt"], prep["npos"]
    rowsum_sim = np.zeros(B)
    S_col = np.zeros(B)
    acc = {name: dict(row=np.zeros(B), col=np.zeros(B))
           for name in ("v", "t")}
    for c in range(NC_CORES):
        r = results[c]
        s = SLAB * c
        gsl = slice(s, s + SLAB)
        rps = r["rp_sim"].astype(np.float64)          # [128, 8, 5]
        # slots 0..3 are written for every m; slot 4 only by the (p=0, m=0)
        # bootstrap half-group — the rest of slot 4 is never written
        rsum = rps[:, :, 0:4].sum(axis=2)
        rsum[:, 0] += rps[:, 0, 4]
        rowsum_sim[gsl] += rsum.T.reshape(SLAB)
        cas = r["ca_sim"].astype(np.float64)          # [4, 128, 2048]
        S_col += np.roll(cas.sum(axis=1).reshape(B), s)
        for name in ("v", "t"):
            rp = r[f"rp_{name}"].astype(np.float64)   # [128, 8, 2]
            rdg = r[f"rd_{name}"].astype(np.float64)  # [128, 8]
            acc[name]["row"][gsl] += (rp.sum(axis=2) + rdg).T.reshape(SLAB)
            ca = r[f"ca_{name}"].astype(np.float64)   # [128, 9*512]
            colfull = np.zeros(B)
            colfull[:9 * NB] = ca.sum(axis=0)
            acc[name]["col"] += np.roll(colfull, s)

    lse_row = np.log(rowsum_sim)
    lse_col = np.log(S_col)
    v2t = (cnt @ lse_row - prep["sig_vt"] * INV_T) / npos
    t2v = (cnt @ lse_col - prep["sig_vt"] * INV_T) / npos

    inst = {}
    for name, sig, diag_raw in (("v", prep["sig_vv"], prep["diag_vv"]),
                                ("t", prep["sig_tt"], prep["diag_tt"])):
        a = acc[name]
        rs = a["row"] + a["col"]
        lse = np.log(rs)
        inst[name] = ((cnt - 1) @ lse - (sig - diag_raw) * INV_T) / npos

    total = 0.5 * (v2t + t2v) + LAMBDA_V * inst["v"] + LAMBDA_T * inst["t"]
    return np.float32(total)


def kernel(vision_features, text_features, match_ids):
    loss, _ = run(vision_features, text_features, match_ids)
    return np.array(loss, dtype=np.float32)
